# revision 1
# baseline (speedup 1.0000x reference)
"""Trainium2 Bass kernel for DistangledMultiHeadAttention.

Data-parallel over batch B=8 across 8 NeuronCores (one batch element per
core).  All matmul datapaths run in bf16 (PSUM accumulation stays fp32 where
accumulation over tiles is needed; single-shot score matmuls write bf16 PSUM
so a whole 1024-wide row block fits one bank).  Inputs/weights are cast to
bf16 host-side, halving DMA bytes and running the PE at full rate.

Per batch (use_adj=1), derived from the reference:
    qhT = 0.125*center_N(Wq^T q^T)          [HD, N]  (bq cancels in centering)
    khT = center_N(Wk^T k^T)                [HD, N]
    vh  = v @ Wv + bv                       [N, HD]  (natural layout; vT is
                                                      the stationary side)
    uT  = softmax_N(Wu^T k^T + bu)          [H, N]
    X_h = (khT_h slice)^T @ qhT_h           [j, i] == x^T   (per head)
    EM  = exp(X) * adjT
    AV  = [vh_h | 1]^T @ EM -> psum [65, i]  (row 64 = s = sum_j EM)
    t2  = (u*vh)^T @ adjT   (per head pair, kept in SBUF)
    ATT^T = AV[:64]/s + t2                   (written straight into SBUF)
    out = ATT @ Wo + bo

Schedule: prefix loads/transposes all inputs, projects v, computes u and t2
(PE-heavy, ACT/DVE light); the main loop interleaves the k/q projections per
head-pair with the attention pipeline (PE) against exp (ACT) and masking
(DVE); output projection last.
"""

import contextlib
import numpy as np
import sys

for _p in ("/opt/trn_rl_repo",):
    if _p not in sys.path:
        sys.path.insert(0, _p)

import ml_dtypes
import concourse.bass as bass
import concourse.mybir as mybir
import concourse.tile as tile
from concourse import bacc
from concourse.masks import make_identity

FP32 = mybir.dt.float32
BF16 = mybir.dt.bfloat16
AF = mybir.ActivationFunctionType
ALU = mybir.AluOpType
P = 128
N, HID, H, D = 1024, 1024, 16, 64
HD = H * D
KO = HID // P
NO = N // P
MO = HD // P
FREE = 512
NIO = N // FREE
HPP = P // D
GS = 4


def build_core_kernel(use_adj=True):
    """Build the single-core Bass program (SPMD: same program on 8 cores)."""
    nc = bacc.Bacc("TRN2", target_bir_lowering=False, debug=False)

    q_d = nc.dram_tensor("q", [N, HID], BF16, kind="ExternalInput")
    k_d = nc.dram_tensor("k", [N, HID], BF16, kind="ExternalInput")
    v_d = nc.dram_tensor("v", [N, HID], BF16, kind="ExternalInput")
    adj_d = nc.dram_tensor("adj", [N, N], BF16, kind="ExternalInput")
    Wq_d = nc.dram_tensor("Wq", [HID, HD], BF16, kind="ExternalInput")
    Wk_d = nc.dram_tensor("Wk", [HID, HD], BF16, kind="ExternalInput")
    Wv_d = nc.dram_tensor("Wv", [HID, HD], BF16, kind="ExternalInput")
    Wu_d = nc.dram_tensor("Wu", [P, KO, H], BF16, kind="ExternalInput")
    Wo_d = nc.dram_tensor("Wo", [HD, HID], BF16, kind="ExternalInput")
    bv_d = nc.dram_tensor("bv", [HD], FP32, kind="ExternalInput")
    bu_d = nc.dram_tensor("bu", [H], FP32, kind="ExternalInput")
    bo_d = nc.dram_tensor("bo", [HID], FP32, kind="ExternalInput")
    out_d = nc.dram_tensor("out", [N, HID], FP32, kind="ExternalOutput")
    r_d = nc.dram_tensor("r_scratch", [H, N], FP32)

    scale = float(D) ** (-0.5)

    with tile.TileContext(nc) as tc:
        with (
            tc.tile_pool(name="persist", bufs=1) as pp,
            tc.tile_pool(name="small", bufs=1) as sp,
            tc.tile_pool(name="meanp", bufs=4) as meanp,
        ):
            ident = sp.tile([P, P], BF16, tag="ident")
            make_identity(nc, ident[:])

            qhT = pp.tile([P, MO, N], BF16, tag="qhT")
            khT = pp.tile([P, MO, N], BF16, tag="khT")
            vha = pp.tile([P, NO, H, D + 1], BF16, tag="vha")
            attT = pp.tile([P, MO, N], BF16, tag="attT")
            WoSB = pp.tile([P, MO, HID], BF16, tag="WoSB")
            kT = pp.tile([P, KO, N], BF16, tag="kT")
            qT = pp.tile([P, KO, N], BF16, tag="qT")
            if use_adj:
                adjT = pp.tile([P, NO, N], BF16, tag="adjT")
                t2sb = pp.tile([P, MO, N], BF16, tag="t2sb")

            bv_bc = sp.tile([P, HD], FP32, tag="bv")
            bo_bc = sp.tile([P, HID], FP32, tag="bo")
            bu_sb = sp.tile([H, 1], FP32, tag="bu")
            nc.sync.dma_start(bv_bc[:], bv_d[None, :].to_broadcast((P, HD)))
            nc.sync.dma_start(bo_bc[:], bo_d[None, :].to_broadcast((P, HID)))
            nc.sync.dma_start(bu_sb[:], bu_d[:, None])

            unaryT = sp.tile([H, N], BF16, tag="unaryT")
            uT = sp.tile([H, N], BF16, tag="uT")
            u_nat = sp.tile([P, NO, H], BF16, tag="u_nat")

            # ones columns of vh_aug
            nc.vector.memset(vha[:, :, :, D], 1.0)

            def load_transposed(src_d, natp, tpsum, dst, evac):
                """Stream src [N, HID] bf16 -> srcT [128, KO, N] bf16 in SBUF."""
                for g in range(NO // GS):
                    rows = []
                    for rj in range(GS):
                        ro = g * GS + rj
                        nat = natp.tile([P, HID], BF16, tag="nat", name="nat")
                        nc.sync.dma_start(nat[:], src_d[ro * P:(ro + 1) * P, :])
                        rows.append(nat)
                    for ko in range(KO):
                        tp = tpsum.tile([P, GS * P], BF16, tag="tp", name="tp")
                        for rj in range(GS):
                            nc.tensor.transpose(
                                tp[:, rj * P:(rj + 1) * P],
                                rows[rj][:, ko * P:(ko + 1) * P], ident[:])
                        evac(dst[:, ko, g * GS * P:(g + 1) * GS * P],
                             tp[:, :GS * P])
                return dst

            act_evac = lambda d, s: nc.scalar.activation(d, s, AF.Copy)
            dve_evac = lambda d, s: nc.vector.tensor_copy(d, s)

            # =============== Prefix: v, adj, k, u, q, t2 ====================
            with (
                tc.tile_pool(name="xTp", bufs=1) as xtp,
                tc.tile_pool(name="nat", bufs=6) as natp,
                tc.tile_pool(name="wvp", bufs=1) as wvp,
                tc.tile_pool(name="wtp", bufs=3) as wtp,
                tc.tile_pool(name="tps", bufs=2, space="PSUM") as tpsum,
                tc.tile_pool(name="vps", bufs=2, space="PSUM") as vpsum,
                tc.tile_pool(name="bps", bufs=2, space="PSUM") as bps,
            ):
                vT = xtp.tile([P, KO, N], BF16, tag="vT")
                load_transposed(v_d, natp, tpsum, vT, act_evac)
                Wv_sb = wvp.tile([P, KO, HD], BF16, tag="Wv")
                nc.sync.dma_start(
                    Wv_sb[:], Wv_d[:].rearrange("(ko p) f -> p ko f", p=P))
                for nb in range(NO):
                    pss = [vpsum.tile([P, FREE], FP32, tag="vp", name="vp")
                           for _ in range(NIO)]
                    for ko in range(KO):
                        for mf in range(NIO):
                            nc.tensor.matmul(
                                pss[mf][:],
                                vT[:, ko, nb * P:(nb + 1) * P],
                                Wv_sb[:, ko, mf * FREE:(mf + 1) * FREE],
                                start=(ko == 0), stop=(ko == KO - 1),
                            )
                    for mf in range(NIO):
                        hh = mf * (H // NIO)
                        nc.vector.tensor_tensor(
                            vha[:, nb, hh:hh + H // NIO, 0:D],
                            pss[mf][:].rearrange("p (h d) -> p h d", d=D),
                            bv_bc[:, mf * FREE:(mf + 1) * FREE]
                            .rearrange("p (h d) -> p h d", d=D),
                            ALU.add)

                if use_adj:
                    load_transposed(adj_d, natp, tpsum, adjT, act_evac)
                load_transposed(k_d, natp, tpsum, kT, act_evac)

                # unary potential (PE) — softmax + u_nat deferred past q's
                # transposes so the small u chain doesn't stall the PE queue
                with tc.tile_pool(name="wup", bufs=1) as wup:
                    Wu_sb = wup.tile([P, KO, H], BF16, tag="Wu")
                    nc.sync.dma_start(Wu_sb[:], Wu_d[:])
                    for io in range(NIO):
                        up = tpsum.tile([P, FREE], FP32, tag="tp", name="up")
                        for ko in range(KO):
                            nc.tensor.matmul(
                                up[0:H, :], Wu_sb[:, ko, :],
                                kT[:, ko, io * FREE:(io + 1) * FREE],
                                start=(ko == 0), stop=(ko == KO - 1))
                        nc.scalar.activation(
                            unaryT[:, io * FREE:(io + 1) * FREE],
                            up[0:H, :], AF.Identity, bias=bu_sb[:])
                usum = sp.tile([H, 1], FP32, tag="usum")
                urec = sp.tile([H, 1], FP32, tag="urec")
                nc.scalar.activation(uT[:], unaryT[:], AF.Exp,
                                     accum_out=usum[:])
                nc.vector.reciprocal(urec[:], usum[:])
                nc.vector.tensor_scalar(uT[:], uT[:], urec[:], None,
                                        op0=ALU.mult)

                load_transposed(q_d, natp, tpsum, qT, act_evac)

                # u_nat [128, NO, H] via PE transposes of uT
                for g in range(NO // GS):
                    tp = tpsum.tile([P, FREE], BF16, tag="tpn", name="tpn")
                    for t in range(GS):
                        no = g * GS + t
                        nc.tensor.transpose(
                            tp[:, t * H:(t + 1) * H],
                            uT[:, no * P:(no + 1) * P], ident[:H, :H])
                    nc.scalar.activation(
                        u_nat[:, g * GS:(g + 1) * GS, :],
                        tp[:, :GS * H].rearrange("p (g h) -> p g h", g=GS),
                        AF.Copy)

                # ---- t2 = (u*vh)^T @ adjT per head pair -> t2sb -------------
                if use_adj:
                    for mo2 in range(MO):
                        pbs = [bps.tile([P, FREE], FP32, tag="pb",
                                        name="pb") for _ in range(NIO)]
                        for jo in range(NO):
                            wt = wtp.tile([P, HPP, D], BF16, tag="wt",
                                          name="wt")
                            nc.vector.tensor_tensor(
                                wt[:],
                                vha[:, jo, mo2 * HPP:(mo2 + 1) * HPP, 0:D],
                                u_nat[:, jo, mo2 * HPP:(mo2 + 1) * HPP,
                                      None].to_broadcast((P, HPP, D)),
                                ALU.mult)
                            for io in range(NIO):
                                nc.tensor.matmul(
                                    pbs[io][:],
                                    wt[:].rearrange("p h d -> p (h d)"),
                                    adjT[:, jo, io * FREE:(io + 1) * FREE],
                                    start=(jo == 0), stop=(jo == NO - 1))
                        for io in range(NIO):
                            nc.scalar.activation(
                                t2sb[:, mo2, io * FREE:(io + 1) * FREE],
                                pbs[io][:], AF.Copy)

            # =============== Main loop: k/q projections + attention =========
            nc.sync.dma_start(
                WoSB[:], Wo_d[:].rearrange("(mo p) f -> p mo f", p=P))
            with (
                tc.tile_pool(name="wkq", bufs=4) as wkq,
                tc.tile_pool(name="emp", bufs=3) as emp,
                tc.tile_pool(name="rbcp", bufs=2) as rbcp,
                tc.tile_pool(name="pjs", bufs=1, space="PSUM") as pjs,
                tc.tile_pool(name="xps", bufs=2, space="PSUM") as xps,
                tc.tile_pool(name="aps", bufs=2, space="PSUM") as aps,
            ):
                def proj(xT, W_d, mo, dst, do_scale):
                    W_sb = wkq.tile([P, KO, P], BF16, tag="wkq", name="W_sb")
                    nc.sync.dma_start(
                        W_sb[:],
                        W_d[:, mo * P:(mo + 1) * P]
                        .rearrange("(ko p) f -> p ko f", p=P))
                    ps = pjs.tile([P, N], FP32, tag="pj", name="ps")
                    for ko in range(KO):
                        for io in range(NIO):
                            nc.tensor.matmul(
                                ps[:, io * FREE:(io + 1) * FREE],
                                W_sb[:, ko, :],
                                xT[:, ko, io * FREE:(io + 1) * FREE],
                                start=(ko == 0), stop=(ko == KO - 1),
                            )
                    sums = meanp.tile([P, 1], FP32, tag="sums", name="sums")
                    nbias = meanp.tile([P, 1], FP32, tag="nbias", name="nbias")
                    nc.vector.tensor_copy(dst[:, mo, :], ps[:])
                    nc.vector.tensor_reduce(sums[:], dst[:, mo, :],
                                            mybir.AxisListType.XYZW, ALU.add)
                    nc.vector.tensor_scalar(
                        nbias[:], sums[:], 1.0 / N, None, op0=ALU.mult)
                    if do_scale:
                        nc.vector.tensor_scalar(
                            dst[:, mo, :], dst[:, mo, :], nbias[:],
                            scale, op0=ALU.subtract, op1=ALU.mult)
                    else:
                        nc.vector.tensor_scalar(
                            dst[:, mo, :], dst[:, mo, :], nbias[:],
                            None, op0=ALU.subtract)

                def head(h):
                    mo, hp = h // HPP, h % HPP
                    psumA = aps.tile([P, N], FP32, tag="pa", name="pa")
                    for jo in range(NO):
                        em = emp.tile([P, N], BF16, tag="em", name="em")
                        for io in range(NIO):
                            xp = xps.tile([P, FREE], FP32, tag="xp",
                                          name="xp")
                            nc.tensor.matmul(
                                xp[:],
                                khT[hp * D:(hp + 1) * D, mo,
                                    jo * P:(jo + 1) * P],
                                qhT[hp * D:(hp + 1) * D, mo,
                                    io * FREE:(io + 1) * FREE],
                                start=True, stop=True)
                            nc.scalar.activation(
                                em[:, io * FREE:(io + 1) * FREE], xp[:],
                                AF.Exp)
                        if use_adj:
                            nc.vector.tensor_tensor(
                                em[:], em[:], adjT[:, jo, :], ALU.mult)
                        for io in range(NIO):
                            nc.tensor.matmul(
                                psumA[0:D + 1, io * FREE:(io + 1) * FREE],
                                vha[:, jo, h, :],
                                em[:, io * FREE:(io + 1) * FREE],
                                start=(jo == 0), stop=(jo == NO - 1))
                    # s row (psum partition 64) -> SBUF, then 1/s, broadcast
                    # via DRAM (SBUF partition-broadcast needs a DMA bounce).
                    s_row = rbcp.tile([1, N], FP32, tag="srow", name="srow")
                    r_row = rbcp.tile([1, N], FP32, tag="rrow", name="rrow")
                    nc.vector.tensor_copy(s_row[:], psumA[D:D + 1, :])
                    nc.vector.reciprocal_approx_fast(r_row[:], s_row[:])
                    r_bc = rbcp.tile([D, N], FP32, tag="rbc", name="rbc")
                    nc.sync.dma_start(r_d[h, None, :], r_row[:])
                    nc.sync.dma_start(
                        r_bc[:], r_d[h, None, :].to_broadcast((D, N)))
                    att = attT[hp * D:(hp + 1) * D, mo, :]
                    nc.vector.tensor_tensor(att, psumA[0:D, :], r_bc[:],
                                            ALU.mult)
                    if use_adj:
                        nc.vector.tensor_tensor(
                            att, att, t2sb[hp * D:(hp + 1) * D, mo, :],
                            ALU.add)
                    else:
                        # u term unmasked is rank-1: t2 = sum_j u_h[j] vh[j,:]
                        t2 = sp.tile([D, 1], FP32, tag=f"t2_{h % 4}",
                                     name="t2")
                        pb1 = xps.tile([P, FREE], FP32, tag="xp", name="pb1")
                        for jo in range(NO):
                            nc.tensor.matmul(
                                pb1[0:D, 0:1], vha[:, jo, h, 0:D],
                                u_nat[:, jo, h, None],
                                start=(jo == 0), stop=(jo == NO - 1))
                        nc.vector.tensor_copy(t2[:], pb1[0:D, 0:1])
                        nc.vector.tensor_tensor(
                            att, att, t2[:].to_broadcast((D, N)), ALU.add)

                for mo in range(MO):
                    proj(kT, Wk_d, mo, khT, False)
                    proj(qT, Wq_d, mo, qhT, True)
                    head(2 * mo)
                    head(2 * mo + 1)

            # =============== Output projection ==============================
            with (
                tc.tile_pool(name="outp", bufs=3) as outp,
                tc.tile_pool(name="ops", bufs=2, space="PSUM") as ops,
            ):
                for ic in range(NO):
                    op = [ops.tile([P, FREE], FP32, tag="op", name="op")
                          for _ in range(NIO)]
                    for mo in range(MO):
                        for mf in range(NIO):
                            nc.tensor.matmul(
                                op[mf][:], attT[:, mo, ic * P:(ic + 1) * P],
                                WoSB[:, mo, mf * FREE:(mf + 1) * FREE],
                                start=(mo == 0), stop=(mo == MO - 1))
                    outt = outp.tile([P, HID], FP32, tag="outt", name="outt")
                    for mf in range(NIO):
                        nc.vector.tensor_tensor(
                            outt[:, mf * FREE:(mf + 1) * FREE], op[mf][:],
                            bo_bc[:, mf * FREE:(mf + 1) * FREE],
                            ALU.add)
                    nc.sync.dma_start(out_d[ic * P:(ic + 1) * P, :], outt[:])

    nc.compile()
    return nc


_CACHE = {}


def _get_nc(use_adj: bool):
    key = bool(use_adj)
    if key not in _CACHE:
        _CACHE[key] = build_core_kernel(use_adj=key)
    return _CACHE[key]


def _make_in_maps(ins=None, **kw):
    if ins is None:
        ins = kw
    BF = ml_dtypes.bfloat16
    gb = lambda n: np.ascontiguousarray(np.asarray(ins[n], np.float32)).astype(BF)
    gf = lambda n: np.ascontiguousarray(np.asarray(ins[n], np.float32))
    q, k, v, adj = gb("q"), gb("k"), gb("v"), gb("adj")
    Wu = np.asarray(ins["Wu"], np.float32)
    shared = {
        "Wq": gb("Wq"), "Wk": gb("Wk"), "Wv": gb("Wv"), "Wo": gb("Wo"),
        "Wu": np.ascontiguousarray(
            Wu.reshape(KO, P, H).transpose(1, 0, 2)).astype(BF),
        "bv": gf("bv"), "bu": gf("bu"), "bo": gf("bo"),
    }
    in_maps = []
    for b in range(q.shape[0]):
        m = dict(shared)
        m["q"], m["k"], m["v"], m["adj"] = q[b], k[b], v[b], adj[b]
        in_maps.append(m)
    return in_maps


def kernel(q, k, v, adj, use_adj, Wq, bq, Wk, bk, Wv, bv, Wu, bu, Wo, bo):
    from concourse.bass_utils import run_bass_kernel_spmd

    nc = _get_nc(bool(int(np.asarray(use_adj))))
    in_maps = _make_in_maps(q=q, k=k, v=v, adj=adj, Wq=Wq, Wk=Wk, Wv=Wv,
                            Wu=Wu, Wo=Wo, bv=bv, bu=bu, bo=bo)
    res = run_bass_kernel_spmd(nc, in_maps, list(range(len(in_maps))))
    return np.stack([res.results[b]["out"] for b in range(len(in_maps))],
                    axis=0)

